# revision 2
# baseline (speedup 1.0000x reference)
"""Trainium2 Bass kernel for a 2-layer GRU (B=64, T=256, IN=128, H=512, OUT=64).

Strategy: data-parallel over batch (8 cores x B_local=8). Each core runs both
GRU layers, interleaved window-by-window, entirely on-core (no collectives).
All tensors are kept "gate-major" (gate/h index on partitions, batch on the
free dim) so the recurrent state h.T feeds the next step's matmuls directly
with no transposes. Weights are pre-transposed/cast to bf16 on the host.

Per layer, gates for a window of WT=8 timesteps are pre-accumulated into a
PSUM window buffer by batched matmuls (x-side GEMM chunks + rank-1 bias
matmuls); the sequential scan then adds W_hh @ h_t per step.

Scheduling notes (from trace analysis):
- Semaphore wait thresholds on the PE completion counter quantize to
  multiples of 16 matmuls, so each step's scan matmuls are grouped
  r(16) -> hn(16) -> z(16) and every emitted block is kept ==0 mod 16
  (window bursts padded with zero rank-1 matmuls). That way sigma(r) can
  issue as soon as the r tiles retire, 1/3 into the block.
- The two layers are software-pipelined: per step tau the emission order is
  [L1 tail(tau-1)] mm0(tau) head0(tau) mm1(tau) tail0(tau) head1(tau),
  which keeps each layer's h-update off the PE's critical path (the PE
  runs the other layer's matmuls while a chain completes).
"""

import sys

sys.path.insert(0, "/opt/trn_rl_repo")

import os
import numpy as np
import ml_dtypes

B, T, IN, H, OUT = 64, 256, 128, 512, 64
T = int(os.environ.get("KT", T))
KDEBUG = os.environ.get("KDEBUG", "0") == "1"
NCORES = 8
BL = B // NCORES          # local batch = 8
WT = 8                    # timesteps per PSUM window
NW = T // WT              # number of windows
G = (3 * H) // 128        # 12 gate tiles of 128
NH = H // 128             # 4 h chunks
BF = ml_dtypes.bfloat16

_COMPILED = None


def _build():
    import concourse.bass as bass
    import concourse.mybir as mybir
    import concourse.tile as tile
    from concourse import bacc

    f32 = mybir.dt.float32
    bf16 = mybir.dt.bfloat16
    ACTF = mybir.ActivationFunctionType
    ALU = mybir.AluOpType

    nc = bacc.Bacc(None, target_bir_lowering=False)

    # ---- I/O ----
    xT_d = nc.dram_tensor("xT", [IN, T * BL], bf16, kind="ExternalInput")
    w0_d = nc.dram_tensor("w0", [128, 60 * 128], bf16, kind="ExternalInput")
    w1_d = nc.dram_tensor("w1", [128, 96 * 128], bf16, kind="ExternalInput")
    b0_d = nc.dram_tensor("b0", [1, 3 * H], bf16, kind="ExternalInput")
    b1_d = nc.dram_tensor("b1", [1, 3 * H], bf16, kind="ExternalInput")
    bhn0_d = nc.dram_tensor("bhn0", [1, H], bf16, kind="ExternalInput")
    bhn1_d = nc.dram_tensor("bhn1", [1, H], bf16, kind="ExternalInput")
    wo_d = nc.dram_tensor("wo", [128, 8 * OUT], bf16, kind="ExternalInput")
    bo_d = nc.dram_tensor("bo", [1, OUT], bf16, kind="ExternalInput")
    out_d = nc.dram_tensor("outT", [OUT, BL], f32, kind="ExternalOutput")
    if KDEBUG:
        h0_dbg = nc.dram_tensor("h0dbg", [128, NH * T * BL], f32, kind="ExternalOutput")
        h1_dbg = nc.dram_tensor("h1dbg", [128, NH * T * BL], f32, kind="ExternalOutput")

    with tile.TileContext(nc) as tc:
        with (
            tc.tile_pool(name="wpool", bufs=1) as wpool,
            tc.tile_pool(name="state", bufs=1) as state,
            tc.tile_pool(name="hist0", bufs=2) as hist0p,
            tc.tile_pool(name="hist1", bufs=2) as hist1p,
            tc.tile_pool(name="tmp", bufs=4) as tmp,
            tc.tile_pool(name="win0", bufs=1, space="PSUM") as win0p,
            tc.tile_pool(name="win1", bufs=1, space="PSUM") as win1p,
            tc.tile_pool(name="headp", bufs=1, space="PSUM") as headp,
        ):
            # ---- load everything to SBUF ----
            xT = wpool.tile([IN, T * BL], bf16)
            w0 = wpool.tile([128, 60, 128], bf16)
            w1 = wpool.tile([128, 96, 128], bf16)
            b0 = wpool.tile([1, 3 * H], bf16)
            b1 = wpool.tile([1, 3 * H], bf16)
            bhn0 = wpool.tile([1, H], bf16)
            bhn1 = wpool.tile([1, H], bf16)
            wo = wpool.tile([128, 8 * OUT], bf16)
            bo = wpool.tile([1, OUT], bf16)
            nc.sync.dma_start(out=xT[:], in_=xT_d[:])
            nc.sync.dma_start(out=w0[:], in_=w0_d[:].rearrange("p (t m) -> p t m", m=128))
            nc.sync.dma_start(out=w1[:], in_=w1_d[:].rearrange("p (t m) -> p t m", m=128))
            nc.sync.dma_start(out=b0[:], in_=b0_d[:])
            nc.sync.dma_start(out=b1[:], in_=b1_d[:])
            nc.sync.dma_start(out=bhn0[:], in_=bhn0_d[:])
            nc.sync.dma_start(out=bhn1[:], in_=bhn1_d[:])
            nc.sync.dma_start(out=wo[:], in_=wo_d[:])
            nc.sync.dma_start(out=bo[:], in_=bo_d[:])

            ones = state.tile([1, WT * BL], bf16)
            nc.vector.memset(ones[:], 1.0)
            zpad = state.tile([1, 128], bf16)
            nc.vector.memset(zpad[:], 0.0)

            # L0 weight tiles: tile 0..11 = W_ih chunk, 12..59 = W_hh (c,g)
            def w0_ih(g):
                return w0[:, g, :]

            def w0_hh(c, g):
                return w0[:, 12 + c * G + g, :]

            # L1: tiles 0..47 = W_ih (c,g), 48..95 = W_hh (c,g)
            def w1_ih(c, g):
                return w1[:, c * G + g, :]

            def w1_hh(c, g):
                return w1[:, 48 + c * G + g, :]

            def emit_window_inputs(lyr, wr, wz, wx, rhs_fn, nk):
                """Pre-fill the three PSUM window tensors for WT timesteps.

                wr/wz: [128, 4, WT*BL] r/z gates. wx: [128, 4, 2*WT*BL] with
                xn in cols [0,WT*BL) and the hn region (pre-filled with the
                n-gate h-side bias) in cols [WT*BL, 2*WT*BL). Each tensor sits
                in its own PSUM bank so gate reads never wait on unrelated
                gate writes. start=True only on the first matmul touching
                each bank. Total matmul count is kept ==0 mod 16 (zero-pad
                rank-1 matmuls) so the PE completion-counter thresholds of
                the scan's consumers stay aligned to real dependencies.
                """
                b_sb = b0 if lyr == 0 else b1
                bhnb = bhn0 if lyr == 0 else bhn1
                count = 0
                for g in range(G):
                    if g < 4:
                        tgt = wr[:, g, :]
                    elif g < 8:
                        tgt = wz[:, g - 4, :]
                    else:
                        tgt = wx[:, g - 8, 0:WT * BL]
                    for c in range(nk):
                        lhsT = w0_ih(g) if lyr == 0 else w1_ih(c, g)
                        nc.tensor.matmul(
                            out=tgt, lhsT=lhsT, rhs=rhs_fn(c),
                            start=(c == 0 and g % 4 == 0), stop=False,
                            skip_group_check=True,
                        )
                        count += 1
                    nc.tensor.matmul(
                        out=tgt, lhsT=b_sb[:, g * 128:(g + 1) * 128],
                        rhs=ones[:], start=False, stop=False,
                        skip_group_check=True,
                    )
                    count += 1
                for g in range(NH):
                    nc.tensor.matmul(
                        out=wx[:, g, WT * BL:2 * WT * BL],
                        lhsT=bhnb[:, g * 128:(g + 1) * 128],
                        rhs=ones[:], start=False, stop=False,
                        skip_group_check=True,
                    )
                    count += 1
                # pad to a multiple of 16 matmuls with zero rank-1 adds
                while count % 16 != 0:
                    nc.tensor.matmul(
                        out=wx[:, 0, WT * BL:2 * WT * BL],
                        lhsT=zpad[:], rhs=ones[:], start=False, stop=False,
                        skip_group_check=True,
                    )
                    count += 1

            def emit_mm(wr, wz, wx, h_prev, tau, whh):
                """One step's 48 scan matmuls in order r, hn, z (16 each)."""
                if h_prev is None:
                    return
                ts = slice(tau * BL, (tau + 1) * BL)
                hs = slice(WT * BL + tau * BL, WT * BL + (tau + 1) * BL)
                for g in range(NH):
                    for c in range(NH):
                        nc.tensor.matmul(
                            out=wr[:, g, ts], lhsT=whh(c, g),
                            rhs=h_prev[:, c, :], start=False,
                            stop=(c == NH - 1), skip_group_check=True,
                        )
                for g in range(NH):
                    for c in range(NH):
                        nc.tensor.matmul(
                            out=wx[:, g, hs], lhsT=whh(c, 8 + g),
                            rhs=h_prev[:, c, :], start=False,
                            stop=(c == NH - 1), skip_group_check=True,
                        )
                for g in range(NH):
                    for c in range(NH):
                        nc.tensor.matmul(
                            out=wz[:, g, ts], lhsT=whh(c, 4 + g),
                            rhs=h_prev[:, c, :], start=False,
                            stop=(c == NH - 1), skip_group_check=True,
                        )

            def emit_head(lyr, wr, wz, wx, tau):
                """Pointwise head: sigma(r), m=r*hn, tt=m+xn, tanh, sigma(z).

                ACT queue order [sigma_r, tanh, sigma_z]: tanh's input is
                ready before wz's tiles retire, sigma_z's consumer (the
                z*d multiply) comes after the tanh-dependent subtract.
                """
                ts = slice(tau * BL, (tau + 1) * BL)
                hs = slice(WT * BL + tau * BL, WT * BL + (tau + 1) * BL)
                sfx = str(lyr)
                r = tmp.tile([128, NH, BL], bf16, tag="r" + sfx)
                z = tmp.tile([128, NH, BL], bf16, tag="z" + sfx)
                n = tmp.tile([128, NH, BL], bf16, tag="n" + sfx)
                tt = tmp.tile([128, NH, BL], mybir.dt.float32, tag="tt" + sfx)
                m = tmp.tile([128, NH, BL], mybir.dt.float32, tag="m" + sfx)
                nc.scalar.activation(r[:], wr[:, :, ts], ACTF.Sigmoid)
                nc.vector.tensor_mul(m[:], r[:], wx[:, :, hs])
                nc.vector.tensor_add(tt[:], m[:], wx[:, :, ts])
                nc.scalar.activation(n[:], tt[:], ACTF.Tanh)
                nc.scalar.activation(z[:], wz[:, :, ts], ACTF.Sigmoid)
                return z, n

            def emit_tail(lyr, h_prev, hist, tau, z, n):
                """h = n + z*(h_prev - n); writes the hist slice for tau."""
                ts = slice(tau * BL, (tau + 1) * BL)
                d = tmp.tile([128, NH, BL], mybir.dt.float32, tag="d" + str(lyr))
                if h_prev is not None:
                    nc.vector.tensor_sub(d[:], h_prev, n[:])
                    nc.vector.tensor_mul(d[:], z[:], d[:])
                    nc.vector.tensor_add(hist[:, :, ts], n[:], d[:])
                else:
                    # t=0: h = n - z*n
                    nc.vector.tensor_mul(d[:], z[:], n[:])
                    nc.vector.tensor_sub(hist[:, :, ts], n[:], d[:])

            # ---- main loop over windows ----
            h0_hist_prev = None
            h1_hist_prev = None
            h1_win_hist = None   # the h0 hist window L1 consumes
            pend1 = None         # (h_prev, hist, tau, z, n) for the L1 tail
            for w in range(NW):
                # L1's pending tail from the previous window's last step must
                # land before this window's L1 burst overwrites wz1/wr1/wx1
                # reads... (the burst waits on the head's reads anyway); emit
                # it first so it leads the DVE queue.
                if pend1 is not None:
                    emit_tail(1, *pend1)
                    pend1 = None
                wr0 = win0p.tile([128, NH, WT * BL], mybir.dt.float32, tag="wr0")
                wz0 = win0p.tile([128, NH, WT * BL], mybir.dt.float32, tag="wz0")
                wx0 = win0p.tile([128, NH, 2 * WT * BL], mybir.dt.float32, tag="wx0")
                h0_hist = hist0p.tile([128, NH, WT * BL], bf16, tag="h0h")
                emit_window_inputs(
                    0, wr0, wz0, wx0, lambda c: xT[:, w * WT * BL:(w + 1) * WT * BL], 1
                )
                if w > 0:
                    wr1 = win1p.tile([128, NH, WT * BL], mybir.dt.float32, tag="wr1")
                    wz1 = win1p.tile([128, NH, WT * BL], mybir.dt.float32, tag="wz1")
                    wx1 = win1p.tile([128, NH, 2 * WT * BL], mybir.dt.float32, tag="wx1")
                    h1_hist = hist1p.tile([128, NH, WT * BL], bf16, tag="h1h")
                    emit_window_inputs(
                        1, wr1, wz1, wx1, lambda c: h1_win_hist[:, c, :], NH
                    )
                for tau in range(WT):
                    if pend1 is not None:
                        emit_tail(1, *pend1)
                        pend1 = None
                    # layer 0, step w*WT + tau
                    if w == 0 and tau == 0:
                        h0_prev = None
                    elif tau == 0:
                        h0_prev = h0_hist_prev[:, :, (WT - 1) * BL:]
                    else:
                        h0_prev = h0_hist[:, :, (tau - 1) * BL:tau * BL]
                    emit_mm(wr0, wz0, wx0, h0_prev, tau, w0_hh)
                    z0, n0 = emit_head(0, wr0, wz0, wx0, tau)
                    # layer 1, step (w-1)*WT + tau (lags one window)
                    if w > 0:
                        if w == 1 and tau == 0:
                            h1_prev = None
                        elif tau == 0:
                            h1_prev = h1_hist_prev[:, :, (WT - 1) * BL:]
                        else:
                            h1_prev = h1_hist[:, :, (tau - 1) * BL:tau * BL]
                        emit_mm(wr1, wz1, wx1, h1_prev, tau, w1_hh)
                    emit_tail(0, h0_prev, h0_hist, tau, z0, n0)
                    if w > 0:
                        z1, n1 = emit_head(1, wr1, wz1, wx1, tau)
                        pend1 = (h1_prev, h1_hist, tau, z1, n1)
                if KDEBUG:
                    sz = NH * WT * BL
                    nc.gpsimd.dma_start(
                        out=h0_dbg[:, w * sz:(w + 1) * sz],
                        in_=h0_hist[:].rearrange("p a b -> p (a b)"))
                    if w > 0:
                        if pend1 is not None:
                            emit_tail(1, *pend1)
                            pend1 = None
                        nc.gpsimd.dma_start(
                            out=h1_dbg[:, (w - 1) * sz:w * sz],
                            in_=h1_hist[:].rearrange("p a b -> p (a b)"))
                h0_hist_prev = h0_hist
                h1_win_hist = h0_hist
                if w > 0:
                    h1_hist_prev = h1_hist

            # final L1 window (consumes last h0 window)
            if pend1 is not None:
                emit_tail(1, *pend1)
                pend1 = None
            wr1 = win1p.tile([128, NH, WT * BL], mybir.dt.float32, tag="wr1")
            wz1 = win1p.tile([128, NH, WT * BL], mybir.dt.float32, tag="wz1")
            wx1 = win1p.tile([128, NH, 2 * WT * BL], mybir.dt.float32, tag="wx1")
            h1_hist = hist1p.tile([128, NH, WT * BL], bf16, tag="h1h")
            emit_window_inputs(1, wr1, wz1, wx1, lambda c: h1_win_hist[:, c, :], NH)
            for tau in range(WT):
                if pend1 is not None:
                    emit_tail(1, *pend1)
                    pend1 = None
                if NW == 1 and tau == 0:
                    h1_prev = None
                elif tau == 0:
                    h1_prev = h1_hist_prev[:, :, (WT - 1) * BL:]
                else:
                    h1_prev = h1_hist[:, :, (tau - 1) * BL:tau * BL]
                emit_mm(wr1, wz1, wx1, h1_prev, tau, w1_hh)
                z1, n1 = emit_head(1, wr1, wz1, wx1, tau)
                pend1 = (h1_prev, h1_hist, tau, z1, n1)
            emit_tail(1, *pend1)
            pend1 = None
            if KDEBUG:
                sz = NH * WT * BL
                nc.gpsimd.dma_start(
                    out=h1_dbg[:, (NW - 1) * sz:NW * sz],
                    in_=h1_hist[:].rearrange("p a b -> p (a b)"))

            # ---- output head: out.T = W_out @ [h0;h1] + b_out ----
            hp = headp.tile([OUT, BL], mybir.dt.float32)
            last = slice((WT - 1) * BL, WT * BL)
            for c in range(NH):
                nc.tensor.matmul(
                    out=hp[:], lhsT=wo[:, c * OUT:(c + 1) * OUT],
                    rhs=h0_hist_prev[:, c, last], start=(c == 0), stop=False,
                    skip_group_check=True,
                )
            for c in range(NH):
                nc.tensor.matmul(
                    out=hp[:], lhsT=wo[:, (NH + c) * OUT:(NH + c + 1) * OUT],
                    rhs=h1_hist[:, c, last], start=False, stop=False,
                    skip_group_check=True,
                )
            nc.tensor.matmul(
                out=hp[:], lhsT=bo[:], rhs=ones[:, 0:BL], start=False, stop=True,
                skip_group_check=True,
            )
            o_sb = state.tile([OUT, BL], mybir.dt.float32)
            nc.vector.tensor_copy(o_sb[:], hp[:])
            nc.sync.dma_start(out=out_d[:], in_=o_sb[:])

    nc.compile()
    return nc


def _prep_inputs(x, W_ih_l0, W_hh_l0, b_ih_l0, b_hh_l0,
                 W_ih_l1, W_hh_l1, b_ih_l1, b_hh_l1, W_out, b_out):
    """Host-side: transpose/cast weights to the kernel's tile layouts."""
    f = np.float32
    # L0 x-side tiles [k, g, m]
    wih0 = W_ih_l0.astype(f).reshape(G, 128, IN).transpose(2, 0, 1)  # [128,12,128]
    whh0 = W_hh_l0.astype(f).reshape(G, 128, NH, 128).transpose(3, 2, 0, 1)  # [k,c,g,m]
    w0 = np.concatenate([wih0.reshape(IN, G, 128),
                         whh0.reshape(128, NH * G, 128)], axis=1)  # [128, 60, 128]
    wih1 = W_ih_l1.astype(f).reshape(G, 128, NH, 128).transpose(3, 2, 0, 1)
    whh1 = W_hh_l1.astype(f).reshape(G, 128, NH, 128).transpose(3, 2, 0, 1)
    w1 = np.concatenate([wih1.reshape(128, NH * G, 128),
                         whh1.reshape(128, NH * G, 128)], axis=1)  # [128, 96, 128]

    bi0, bh0 = b_ih_l0.astype(f), b_hh_l0.astype(f)
    bi1, bh1 = b_ih_l1.astype(f), b_hh_l1.astype(f)
    # window bias: r,z gates get b_ih+b_hh; n gates get b_ih only
    b0 = np.concatenate([(bi0 + bh0)[:2 * H], bi0[2 * H:]])
    b1 = np.concatenate([(bi1 + bh1)[:2 * H], bi1[2 * H:]])
    # n-gate h-side bias, tile layout [128, NH]
    bhn0 = bh0[2 * H:].reshape(1, H)
    bhn1 = bh1[2 * H:].reshape(1, H)
    # head: wo[k, c*OUT+m] = W_out[m, c*128+k]
    wo = W_out.astype(f).reshape(OUT, 8, 128).transpose(2, 1, 0).reshape(128, 8 * OUT)

    common = {
        "w0": w0.reshape(128, 60 * 128).astype(BF),
        "w1": w1.reshape(128, 96 * 128).astype(BF),
        "b0": b0.reshape(1, 3 * H).astype(BF),
        "b1": b1.reshape(1, 3 * H).astype(BF),
        "bhn0": bhn0.astype(BF),
        "bhn1": bhn1.astype(BF),
        "wo": wo.astype(BF),
        "bo": b_out.astype(f).reshape(1, OUT).astype(BF),
    }
    in_maps = []
    for c in range(NCORES):
        xs = np.asarray(x[c * BL:(c + 1) * BL, :T], dtype=f)  # [BL, T, IN]
        xT = np.ascontiguousarray(xs.transpose(2, 1, 0)).reshape(IN, T * BL)
        in_maps.append({"xT": xT.astype(BF), **common})
    return in_maps


TRACE = False
LAST_RESULT = None


def kernel(**inputs):
    global _COMPILED, LAST_RESULT
    from concourse.bass_utils import run_bass_kernel_spmd

    if _COMPILED is None:
        _COMPILED = _build()
    nc = _COMPILED
    in_maps = _prep_inputs(**{k: np.asarray(v) for k, v in inputs.items()})
    res = run_bass_kernel_spmd(nc, in_maps, list(range(NCORES)), trace=TRACE)
    LAST_RESULT = res
    out = np.empty((B, OUT), np.float32)
    for c in range(NCORES):
        out[c * BL:(c + 1) * BL] = res.results[c]["outT"].T
    return out


# revision 7
# speedup vs baseline: 1.2509x; 1.2509x over previous
"""Trainium2 Bass kernel for a 2-layer GRU (B=64, T=256, IN=128, H=512, OUT=64).

Strategy: data-parallel over batch (8 cores x B_local=8). Each core runs both
GRU layers, interleaved window-by-window, entirely on-core (no collectives).
All tensors are kept "gate-major" (gate/h index on partitions, batch on the
free dim) so the recurrent state h.T feeds the next step's matmuls directly
with no transposes. Weights are pre-transposed/cast to bf16 on the host.

Per layer, gates for a window of WT=8 timesteps are pre-accumulated into a
PSUM window buffer by batched matmuls (x-side GEMM chunks + rank-1 bias
matmuls); the sequential scan then adds W_hh @ h_t per step.

Scheduling notes (from trace analysis):
- Semaphore wait thresholds on the PE completion counter quantize to
  multiples of 16 matmuls, so each step's scan matmuls are grouped
  r(16) -> hn(16) -> z(16) and every emitted block is kept ==0 mod 16
  (window bursts padded with zero rank-1 matmuls). That way sigma(r) can
  issue as soon as the r tiles retire, 1/3 into the block.
- The two layers are software-pipelined: per step tau the emission order is
  [L1 tail(tau-1)] mm0(tau) head0(tau) mm1(tau) tail0(tau) head1(tau),
  which keeps each layer's h-update off the PE's critical path (the PE
  runs the other layer's matmuls while a chain completes).
"""

import sys

sys.path.insert(0, "/opt/trn_rl_repo")

import os
import numpy as np
import ml_dtypes

B, T, IN, H, OUT = 64, 256, 128, 512, 64
T = int(os.environ.get("KT", T))
KDEBUG = os.environ.get("KDEBUG", "0") == "1"
NCORES = 8
BL = B // NCORES          # local batch = 8
WT = 8                    # timesteps per PSUM window
NW = T // WT              # number of windows
G = (3 * H) // 128        # 12 gate tiles of 128
NH = H // 128             # 4 h chunks
BF = ml_dtypes.bfloat16

_COMPILED = None


def _build():
    import concourse.bass as bass
    import concourse.mybir as mybir
    import concourse.tile as tile
    from concourse import bacc

    f32 = mybir.dt.float32
    bf16 = mybir.dt.bfloat16
    ACTF = mybir.ActivationFunctionType
    ALU = mybir.AluOpType

    nc = bacc.Bacc(None, target_bir_lowering=False)

    # ---- I/O ----
    xT_d = nc.dram_tensor("xT", [IN, T * BL], bf16, kind="ExternalInput")
    w0_d = nc.dram_tensor("w0", [128, 60 * 128], bf16, kind="ExternalInput")
    w1_d = nc.dram_tensor("w1", [128, 96 * 128], bf16, kind="ExternalInput")
    b0_d = nc.dram_tensor("b0", [1, 3 * H], bf16, kind="ExternalInput")
    b1_d = nc.dram_tensor("b1", [1, 3 * H], bf16, kind="ExternalInput")
    bhn0_d = nc.dram_tensor("bhn0", [1, H], bf16, kind="ExternalInput")
    bhn1_d = nc.dram_tensor("bhn1", [1, H], bf16, kind="ExternalInput")
    wo_d = nc.dram_tensor("wo", [128, 8 * OUT], bf16, kind="ExternalInput")
    bo_d = nc.dram_tensor("bo", [1, OUT], bf16, kind="ExternalInput")
    out_d = nc.dram_tensor("outT", [OUT, BL], f32, kind="ExternalOutput")
    if KDEBUG:
        h0_dbg = nc.dram_tensor("h0dbg", [128, NH * T * BL], f32, kind="ExternalOutput")
        h1_dbg = nc.dram_tensor("h1dbg", [128, NH * T * BL], f32, kind="ExternalOutput")

    with tile.TileContext(nc) as tc:
        with (
            tc.tile_pool(name="wpool", bufs=1) as wpool,
            tc.tile_pool(name="state", bufs=1) as state,
            tc.tile_pool(name="hist0", bufs=2) as hist0p,
            tc.tile_pool(name="hist1", bufs=2) as hist1p,
            tc.tile_pool(name="tmp", bufs=4) as tmp,
            tc.tile_pool(name="win0", bufs=1, space="PSUM") as win0p,
            tc.tile_pool(name="win1", bufs=1, space="PSUM") as win1p,
            tc.tile_pool(name="headp", bufs=1, space="PSUM") as headp,
        ):
            # ---- load everything to SBUF ----
            xT = wpool.tile([IN, T * BL], bf16)
            w0 = wpool.tile([128, 60, 128], bf16)
            w1 = wpool.tile([128, 96, 128], bf16)
            b0 = wpool.tile([1, 3 * H], bf16)
            b1 = wpool.tile([1, 3 * H], bf16)
            bhn0 = wpool.tile([1, H], bf16)
            bhn1 = wpool.tile([1, H], bf16)
            wo = wpool.tile([128, 8 * OUT], bf16)
            bo = wpool.tile([1, OUT], bf16)
            nc.sync.dma_start(out=xT[:], in_=xT_d[:])
            nc.sync.dma_start(out=w0[:], in_=w0_d[:].rearrange("p (t m) -> p t m", m=128))
            nc.sync.dma_start(out=w1[:], in_=w1_d[:].rearrange("p (t m) -> p t m", m=128))
            nc.sync.dma_start(out=b0[:], in_=b0_d[:])
            nc.sync.dma_start(out=b1[:], in_=b1_d[:])
            nc.sync.dma_start(out=bhn0[:], in_=bhn0_d[:])
            nc.sync.dma_start(out=bhn1[:], in_=bhn1_d[:])
            nc.sync.dma_start(out=wo[:], in_=wo_d[:])
            nc.sync.dma_start(out=bo[:], in_=bo_d[:])

            ones = state.tile([1, WT * BL], bf16)
            nc.vector.memset(ones[:], 1.0)
            zpad = state.tile([1, 128], bf16)
            nc.vector.memset(zpad[:], 0.0)

            # Logical scheduling clock. The Tile scheduler orders each
            # engine's queue with a CoreSim whose matmul cost model is ~30x
            # too fast (LDWEIGHTS unmodeled), so left alone it hoists every
            # matmul-fed ACT/DVE op ahead of the chain-fed ones, bunching
            # both layers' tanh/h-update chains into a serial tail (2us of
            # PE idle per step). bass_wait_until_ts floors pin the intended
            # interleave; slot spacing (300ns) exceeds the sim's own ACT/DVE
            # latencies so floor order == dispatch order in-sim. On HW the
            # floors vanish; pacing comes from the data-dep semaphores.
            LCLK = [0.0]

            def tick(n=1.0):
                LCLK[0] += n * 0.0003  # ms units; 1 tick = 300ns of sim time
                tc.tile_set_cur_wait(LCLK[0])

            # L0 weight tiles: tile 0..11 = W_ih chunk, 12..59 = W_hh (c,g)
            def w0_ih(g):
                return w0[:, g, :]

            def w0_hh(c, g):
                return w0[:, 12 + c * G + g, :]

            # L1: tiles 0..47 = W_ih (c,g), 48..95 = W_hh (c,g)
            def w1_ih(c, g):
                return w1[:, c * G + g, :]

            def w1_hh(c, g):
                return w1[:, 48 + c * G + g, :]

            def emit_window_inputs(lyr, wr, wz, wx, rhs_fn, nk):
                """Pre-fill the three PSUM window tensors for WT timesteps.

                wr/wz: [128, 4, WT*BL] r/z gates. wx: [128, 4, 2*WT*BL] with
                xn in cols [0,WT*BL) and the hn region (pre-filled with the
                n-gate h-side bias) in cols [WT*BL, 2*WT*BL). Each tensor sits
                in its own PSUM bank so gate reads never wait on unrelated
                gate writes. start=True only on the first matmul touching
                each bank. Total matmul count is kept ==0 mod 16 (zero-pad
                rank-1 matmuls) so the PE completion-counter thresholds of
                the scan's consumers stay aligned to real dependencies.
                """
                b_sb = b0 if lyr == 0 else b1
                bhnb = bhn0 if lyr == 0 else bhn1
                tick()
                count = 0
                for g in range(G):
                    if g < 4:
                        tgt = wr[:, g, :]
                    elif g < 8:
                        tgt = wz[:, g - 4, :]
                    else:
                        tgt = wx[:, g - 8, 0:WT * BL]
                    for c in range(nk):
                        lhsT = w0_ih(g) if lyr == 0 else w1_ih(c, g)
                        nc.tensor.matmul(
                            out=tgt, lhsT=lhsT, rhs=rhs_fn(c),
                            start=(c == 0 and g % 4 == 0), stop=False,
                            skip_group_check=True,
                        )
                        count += 1
                    nc.tensor.matmul(
                        out=tgt, lhsT=b_sb[:, g * 128:(g + 1) * 128],
                        rhs=ones[:], start=False, stop=False,
                        skip_group_check=True,
                    )
                    count += 1
                for g in range(NH):
                    nc.tensor.matmul(
                        out=wx[:, g, WT * BL:2 * WT * BL],
                        lhsT=bhnb[:, g * 128:(g + 1) * 128],
                        rhs=ones[:], start=False, stop=False,
                        skip_group_check=True,
                    )
                    count += 1
                # pad to a multiple of 16 matmuls with zero rank-1 adds
                while count % 16 != 0:
                    nc.tensor.matmul(
                        out=wx[:, 0, WT * BL:2 * WT * BL],
                        lhsT=zpad[:], rhs=ones[:], start=False, stop=False,
                        skip_group_check=True,
                    )
                    count += 1

            def emit_mm(wr, wz, wx, h_prev, tau, whh):
                """One step's 48 scan matmuls in order r, hn, z (16 each)."""
                if h_prev is None:
                    return
                tick()
                ts = slice(tau * BL, (tau + 1) * BL)
                hs = slice(WT * BL + tau * BL, WT * BL + (tau + 1) * BL)
                for g in range(NH):
                    for c in range(NH):
                        nc.tensor.matmul(
                            out=wr[:, g, ts], lhsT=whh(c, g),
                            rhs=h_prev[:, c, :], start=False,
                            stop=(c == NH - 1), skip_group_check=True,
                        )
                for g in range(NH):
                    for c in range(NH):
                        nc.tensor.matmul(
                            out=wx[:, g, hs], lhsT=whh(c, 8 + g),
                            rhs=h_prev[:, c, :], start=False,
                            stop=(c == NH - 1), skip_group_check=True,
                        )
                for g in range(NH):
                    for c in range(NH):
                        nc.tensor.matmul(
                            out=wz[:, g, ts], lhsT=whh(c, 4 + g),
                            rhs=h_prev[:, c, :], start=False,
                            stop=(c == NH - 1), skip_group_check=True,
                        )

            def emit_head(lyr, wr, wz, wx, tau):
                """Pointwise head: sigma(r), m=r*hn, tt=m+xn, tanh, sigma(z).

                ACT queue order [sigma_r, tanh, sigma_z]: tanh's input is
                ready before wz's tiles retire, sigma_z's consumer (the
                z*d multiply) comes after the tanh-dependent subtract.
                """
                ts = slice(tau * BL, (tau + 1) * BL)
                hs = slice(WT * BL + tau * BL, WT * BL + (tau + 1) * BL)
                sfx = str(lyr)
                r = tmp.tile([128, NH, BL], bf16, tag="r" + sfx)
                z = tmp.tile([128, NH, BL], bf16, tag="z" + sfx)
                n = tmp.tile([128, NH, BL], bf16, tag="n" + sfx)
                tt = tmp.tile([128, NH, BL], mybir.dt.float32, tag="tt" + sfx)
                m = tmp.tile([128, NH, BL], mybir.dt.float32, tag="m" + sfx)
                tick()
                nc.scalar.activation(r[:], wr[:, :, ts], ACTF.Sigmoid)
                tick()
                nc.vector.tensor_mul(m[:], r[:], wx[:, :, hs])
                tick()
                nc.vector.tensor_add(tt[:], m[:], wx[:, :, ts])
                tick()
                nc.scalar.activation(n[:], tt[:], ACTF.Tanh)
                tick()
                nc.scalar.activation(z[:], wz[:, :, ts], ACTF.Sigmoid)
                return z, n

            def emit_tail(lyr, h_prev, hist, tau, z, n):
                """h = n + z*(h_prev - n); writes the hist slice for tau."""
                ts = slice(tau * BL, (tau + 1) * BL)
                d = tmp.tile([128, NH, BL], mybir.dt.float32, tag="d" + str(lyr))
                if h_prev is not None:
                    tick()
                    nc.vector.tensor_sub(d[:], h_prev, n[:])
                    tick()
                    nc.vector.tensor_mul(d[:], z[:], d[:])
                    tick()
                    nc.vector.tensor_add(hist[:, :, ts], n[:], d[:])
                else:
                    # t=0: h = n - z*n
                    tick()
                    nc.vector.tensor_mul(d[:], z[:], n[:])
                    tick()
                    nc.vector.tensor_sub(hist[:, :, ts], n[:], d[:])

            # ---- main loop over windows ----
            h0_hist_prev = None
            h1_hist_prev = None
            h1_win_hist = None   # the h0 hist window L1 consumes
            pend1 = None         # (h_prev, hist, tau, z, n) for the L1 tail
            for w in range(NW):
                # L1's pending tail from the previous window's last step must
                # land before this window's L1 burst overwrites wz1/wr1/wx1
                # reads... (the burst waits on the head's reads anyway); emit
                # it first so it leads the DVE queue.
                if pend1 is not None:
                    emit_tail(1, *pend1)
                    pend1 = None
                wr0 = win0p.tile([128, NH, WT * BL], mybir.dt.float32, tag="wr0")
                wz0 = win0p.tile([128, NH, WT * BL], mybir.dt.float32, tag="wz0")
                wx0 = win0p.tile([128, NH, 2 * WT * BL], mybir.dt.float32, tag="wx0")
                h0_hist = hist0p.tile([128, NH, WT * BL], bf16, tag="h0h")
                emit_window_inputs(
                    0, wr0, wz0, wx0, lambda c: xT[:, w * WT * BL:(w + 1) * WT * BL], 1
                )
                if w > 0:
                    wr1 = win1p.tile([128, NH, WT * BL], mybir.dt.float32, tag="wr1")
                    wz1 = win1p.tile([128, NH, WT * BL], mybir.dt.float32, tag="wz1")
                    wx1 = win1p.tile([128, NH, 2 * WT * BL], mybir.dt.float32, tag="wx1")
                    h1_hist = hist1p.tile([128, NH, WT * BL], bf16, tag="h1h")
                    emit_window_inputs(
                        1, wr1, wz1, wx1, lambda c: h1_win_hist[:, c, :], NH
                    )
                for tau in range(WT):
                    if pend1 is not None:
                        emit_tail(1, *pend1)
                        pend1 = None
                    # layer 0, step w*WT + tau
                    if w == 0 and tau == 0:
                        h0_prev = None
                    elif tau == 0:
                        h0_prev = h0_hist_prev[:, :, (WT - 1) * BL:]
                    else:
                        h0_prev = h0_hist[:, :, (tau - 1) * BL:tau * BL]
                    emit_mm(wr0, wz0, wx0, h0_prev, tau, w0_hh)
                    z0, n0 = emit_head(0, wr0, wz0, wx0, tau)
                    # layer 1, step (w-1)*WT + tau (lags one window)
                    if w > 0:
                        if w == 1 and tau == 0:
                            h1_prev = None
                        elif tau == 0:
                            h1_prev = h1_hist_prev[:, :, (WT - 1) * BL:]
                        else:
                            h1_prev = h1_hist[:, :, (tau - 1) * BL:tau * BL]
                        emit_mm(wr1, wz1, wx1, h1_prev, tau, w1_hh)
                    emit_tail(0, h0_prev, h0_hist, tau, z0, n0)
                    if w > 0:
                        z1, n1 = emit_head(1, wr1, wz1, wx1, tau)
                        pend1 = (h1_prev, h1_hist, tau, z1, n1)
                if KDEBUG:
                    sz = NH * WT * BL
                    nc.gpsimd.dma_start(
                        out=h0_dbg[:, w * sz:(w + 1) * sz],
                        in_=h0_hist[:].rearrange("p a b -> p (a b)"))
                    if w > 0:
                        if pend1 is not None:
                            emit_tail(1, *pend1)
                            pend1 = None
                        nc.gpsimd.dma_start(
                            out=h1_dbg[:, (w - 1) * sz:w * sz],
                            in_=h1_hist[:].rearrange("p a b -> p (a b)"))
                h0_hist_prev = h0_hist
                h1_win_hist = h0_hist
                if w > 0:
                    h1_hist_prev = h1_hist

            # final L1 window (consumes last h0 window)
            if pend1 is not None:
                emit_tail(1, *pend1)
                pend1 = None
            wr1 = win1p.tile([128, NH, WT * BL], mybir.dt.float32, tag="wr1")
            wz1 = win1p.tile([128, NH, WT * BL], mybir.dt.float32, tag="wz1")
            wx1 = win1p.tile([128, NH, 2 * WT * BL], mybir.dt.float32, tag="wx1")
            h1_hist = hist1p.tile([128, NH, WT * BL], bf16, tag="h1h")
            emit_window_inputs(1, wr1, wz1, wx1, lambda c: h1_win_hist[:, c, :], NH)
            for tau in range(WT):
                if pend1 is not None:
                    emit_tail(1, *pend1)
                    pend1 = None
                if NW == 1 and tau == 0:
                    h1_prev = None
                elif tau == 0:
                    h1_prev = h1_hist_prev[:, :, (WT - 1) * BL:]
                else:
                    h1_prev = h1_hist[:, :, (tau - 1) * BL:tau * BL]
                emit_mm(wr1, wz1, wx1, h1_prev, tau, w1_hh)
                z1, n1 = emit_head(1, wr1, wz1, wx1, tau)
                pend1 = (h1_prev, h1_hist, tau, z1, n1)
            emit_tail(1, *pend1)
            pend1 = None
            if KDEBUG:
                sz = NH * WT * BL
                nc.gpsimd.dma_start(
                    out=h1_dbg[:, (NW - 1) * sz:NW * sz],
                    in_=h1_hist[:].rearrange("p a b -> p (a b)"))

            # ---- output head: out.T = W_out @ [h0;h1] + b_out ----
            hp = headp.tile([OUT, BL], mybir.dt.float32)
            last = slice((WT - 1) * BL, WT * BL)
            for c in range(NH):
                nc.tensor.matmul(
                    out=hp[:], lhsT=wo[:, c * OUT:(c + 1) * OUT],
                    rhs=h0_hist_prev[:, c, last], start=(c == 0), stop=False,
                    skip_group_check=True,
                )
            for c in range(NH):
                nc.tensor.matmul(
                    out=hp[:], lhsT=wo[:, (NH + c) * OUT:(NH + c + 1) * OUT],
                    rhs=h1_hist[:, c, last], start=False, stop=False,
                    skip_group_check=True,
                )
            nc.tensor.matmul(
                out=hp[:], lhsT=bo[:], rhs=ones[:, 0:BL], start=False, stop=True,
                skip_group_check=True,
            )
            o_sb = state.tile([OUT, BL], mybir.dt.float32)
            nc.vector.tensor_copy(o_sb[:], hp[:])
            nc.sync.dma_start(out=out_d[:], in_=o_sb[:])

    nc.compile()
    return nc


def _prep_inputs(x, W_ih_l0, W_hh_l0, b_ih_l0, b_hh_l0,
                 W_ih_l1, W_hh_l1, b_ih_l1, b_hh_l1, W_out, b_out):
    """Host-side: transpose/cast weights to the kernel's tile layouts."""
    f = np.float32
    # L0 x-side tiles [k, g, m]
    wih0 = W_ih_l0.astype(f).reshape(G, 128, IN).transpose(2, 0, 1)  # [128,12,128]
    whh0 = W_hh_l0.astype(f).reshape(G, 128, NH, 128).transpose(3, 2, 0, 1)  # [k,c,g,m]
    w0 = np.concatenate([wih0.reshape(IN, G, 128),
                         whh0.reshape(128, NH * G, 128)], axis=1)  # [128, 60, 128]
    wih1 = W_ih_l1.astype(f).reshape(G, 128, NH, 128).transpose(3, 2, 0, 1)
    whh1 = W_hh_l1.astype(f).reshape(G, 128, NH, 128).transpose(3, 2, 0, 1)
    w1 = np.concatenate([wih1.reshape(128, NH * G, 128),
                         whh1.reshape(128, NH * G, 128)], axis=1)  # [128, 96, 128]

    bi0, bh0 = b_ih_l0.astype(f), b_hh_l0.astype(f)
    bi1, bh1 = b_ih_l1.astype(f), b_hh_l1.astype(f)
    # window bias: r,z gates get b_ih+b_hh; n gates get b_ih only
    b0 = np.concatenate([(bi0 + bh0)[:2 * H], bi0[2 * H:]])
    b1 = np.concatenate([(bi1 + bh1)[:2 * H], bi1[2 * H:]])
    # n-gate h-side bias, tile layout [128, NH]
    bhn0 = bh0[2 * H:].reshape(1, H)
    bhn1 = bh1[2 * H:].reshape(1, H)
    # head: wo[k, c*OUT+m] = W_out[m, c*128+k]
    wo = W_out.astype(f).reshape(OUT, 8, 128).transpose(2, 1, 0).reshape(128, 8 * OUT)

    common = {
        "w0": w0.reshape(128, 60 * 128).astype(BF),
        "w1": w1.reshape(128, 96 * 128).astype(BF),
        "b0": b0.reshape(1, 3 * H).astype(BF),
        "b1": b1.reshape(1, 3 * H).astype(BF),
        "bhn0": bhn0.astype(BF),
        "bhn1": bhn1.astype(BF),
        "wo": wo.astype(BF),
        "bo": b_out.astype(f).reshape(1, OUT).astype(BF),
    }
    in_maps = []
    for c in range(NCORES):
        xs = np.asarray(x[c * BL:(c + 1) * BL, :T], dtype=f)  # [BL, T, IN]
        xT = np.ascontiguousarray(xs.transpose(2, 1, 0)).reshape(IN, T * BL)
        in_maps.append({"xT": xT.astype(BF), **common})
    return in_maps


TRACE = False
LAST_RESULT = None


def kernel(**inputs):
    global _COMPILED, LAST_RESULT
    from concourse.bass_utils import run_bass_kernel_spmd

    if _COMPILED is None:
        _COMPILED = _build()
    nc = _COMPILED
    in_maps = _prep_inputs(**{k: np.asarray(v) for k, v in inputs.items()})
    res = run_bass_kernel_spmd(nc, in_maps, list(range(NCORES)), trace=TRACE)
    LAST_RESULT = res
    out = np.empty((B, OUT), np.float32)
    for c in range(NCORES):
        out[c * BL:(c + 1) * BL] = res.results[c]["outT"].T
    return out
